# revision 15
# baseline (speedup 1.0000x reference)
"""DCT heat-blur kernel for Trainium2 (8 NeuronCores, Bass/Tile).

Reference computes, per image X:  out = M X M^T  with  M = D diag(e) D
(e_k = exp(-(pi k/N)^2 sigma^2/2); the reference's "inverse" einsum applies
D again, not D^T).

Spectral truncation: e_k decays fast, so truncate the contraction at K where
e_k < ~1e-3:  M ~= F^T G  with  G = diag(sqrt(e)) D[:K]  (K x N)  and
F = diag(sqrt(e)) D^T[:K]  (K x N), giving  out = F^T (G X G^T) F.

Device chain (matmul computes lhsT.T @ rhs, contracting partitions):
  pass 1 (analysis, truncated):
    s1: T1_c = X_c G^T      lhsT = X_c^T chunks (host-transposed), rhs = G^T
    s2: C_c  = G X_c G^T    lhsT = G^T chunk (const!), rhs = [T1_0|T1_1|T1_2]
        -> 3 channels in one matmul pair; C_c (K x K) parked in SBUF fp16
  pass 2 (synthesis, dense N=256 streams):
    s3: Q_c = C_c^T F       lhsT = C_c, rhs = F
    s4: out = Q_c^T F (= F^T C_c F)

Batches with K > 128 (small sigma) use the dense 2-GEMM path with
W = M^T per batch (out = W^T X W), processed first to warm the PE.

Sharding: batches sorted by fwd_step, dealt into 16 blocks of 8; core i
takes the i-th batch of each block, so all 8 cores run ONE program whose
per-slot K is the block max.  The program is compiled (and cached) per
slot-plan derived from the runtime fwd_steps.

Heavily-blurred slots ship x as fp8e4m3 (SWDGE DMA casts to fp16 in
flight).  Output is stored fp16 (tolerance 2e-2).
"""

import os
import numpy as np
import ml_dtypes

BATCH = 128
CHANNELS = 3
N = 256
N_CORES = 8
NSLOT = 16
IM = CHANNELS * 2 * N

USE_FP8 = os.environ.get("BASS_DCT_FP8", "0") == "1"
FP8_MIN_STEP = int(os.environ.get("BASS_DCT_FP8_MIN_STEP", "6"))
KCAP = 128
K_COEF = float(os.environ.get("BASS_DCT_K_COEF", "604"))

LAST_EXEC_TIME_NS = None
_NC_CACHE = {}
_CONST_CACHE = {}


def _k_of_step(s):
    k = int(np.ceil(K_COEF / (s + 1.0) / 16.0) * 16)
    return max(32, k)


def _slot_plan(steps):
    order = np.argsort(steps, kind="stable")
    slots = []
    for j in range(NSLOT):
        smin = int(steps[order[8 * j]])
        k = _k_of_step(smin)
        if k > KCAP:
            slots.append(("W", 0, False))
        else:
            slots.append(("G", k, USE_FP8 and smin >= FP8_MIN_STEP))
    return order, slots


def _install_ntff_hook():
    import sys
    import types

    if "antenv.axon_hooks" in sys.modules:
        return
    try:
        import trn_agent_boot.trn_boot as tb

        hook = tb._ntff_profile_via_ctypes("/opt/axon/libaxon_pjrt.so")
    except Exception:
        hook = None
    m = types.ModuleType("antenv.axon_hooks")
    m.get_axon_ntff_profile_hook = lambda: hook
    m.set_axon_ntff_profile_hook = lambda h: None
    sys.modules["antenv.axon_hooks"] = m


def _layout(slots):
    """Shared layout: processing order j ascending (W slots first)."""
    proc = list(range(NSLOT))
    xpos = {}
    n16 = n8 = 0
    coff = {}
    ccols = 0
    for j in proc:
        mode, K, is8 = slots[j]
        if is8:
            xpos[j] = n8
            n8 += 1
        else:
            xpos[j] = n16
            n16 += 1
        if mode == "W":
            coff[j] = ("W", ccols, 2 * N)
            ccols += 2 * N
        else:
            coff[j] = ("G", ccols, 2 * K + N)
            ccols += 2 * K + N
    # const chunks: first chunk = first 2 slots (small, early), rest in 3
    groups = [proc[0:2], proc[2:7], proc[7:12], proc[12:16]]
    bounds = [0]
    acc = 0
    for g in groups:
        acc += sum(coff[j][2] for j in g)
        bounds.append(acc)
    return proc, xpos, n16, n8, coff, ccols, bounds


def _build_nc(key):
    import concourse.bacc as bacc
    import concourse.tile as tile
    import concourse.mybir as mybir

    f32 = mybir.dt.float32
    f16 = mybir.dt.float16
    f8 = mybir.dt.float8e4

    slots = list(key)
    proc, xpos, n16, n8, coff, ccols, bounds = _layout(slots)
    gslots = [j for j in proc if slots[j][0] == "G"]
    wslots = [j for j in proc if slots[j][0] == "W"]

    nc = bacc.Bacc("TRN2", target_bir_lowering=False, debug=False)
    x16_d = (
        nc.dram_tensor("x16", [n16, 128, IM], f16, kind="ExternalInput").ap()
        if n16
        else None
    )
    x8_d = (
        nc.dram_tensor("x8", [n8, 128, IM], f8, kind="ExternalInput").ap()
        if n8
        else None
    )
    gw_d = nc.dram_tensor("gw", [128, ccols], f16, kind="ExternalInput").ap()
    o_d = nc.dram_tensor("o", [NSLOT, 128, IM], f16, kind="ExternalOutput").ap()

    def oview(j):
        return o_d[j].rearrange("p (c a w) -> p c a w", c=CHANNELS, a=2)

    with tile.TileContext(nc) as tc:
        with (
            tc.tile_pool(name="const", bufs=1) as cpool,
            tc.tile_pool(name="xpool", bufs=NSLOT + 1) as xpool,
            tc.tile_pool(name="t1p", bufs=6) as t1pool,
            tc.tile_pool(name="csp", bufs=NSLOT + 1) as cspool,
            tc.tile_pool(name="q3p", bufs=9) as q3pool,
            tc.tile_pool(name="opool", bufs=6) as opool,
            tc.tile_pool(name="ps1", bufs=4, space="PSUM") as ps1,
            tc.tile_pool(name="ps2", bufs=4, space="PSUM") as ps2,
        ):
            ctiles = []
            for q in range(4):
                lo, hi = bounds[q], bounds[q + 1]
                if hi == lo:
                    ctiles.append(None)
                    continue
                ct = cpool.tile([128, hi - lo], f16, name=f"gw{q}")
                if q == 0:
                    with tc.high_priority():
                        nc.sync.dma_start(ct[:], gw_d[:, lo:hi])
                else:
                    (nc.scalar if q % 2 else nc.sync).dma_start(
                        ct[:], gw_d[:, lo:hi]
                    )
                ctiles.append(ct)

            def cslice(j):
                base = coff[j][1]
                q = 0
                while bounds[q + 1] <= base:
                    q += 1
                return ctiles[q], base - bounds[q]

            xts = {}
            for j in proc:
                mode, K, is8 = slots[j]
                xt = xpool.tile([128, CHANNELS, 2, N], f16, tag="xt")
                src = (x8_d if is8 else x16_d)[xpos[j]].rearrange(
                    "p (c a w) -> p c a w", c=CHANNELS, a=2
                )
                (nc.gpsimd if is8 else nc.sync).dma_start(xt[:], src)
                xts[j] = xt

            eng_load = [0.0, 0.0]  # estimated busy ns: [DVE, ACT]

            def copy(dst, src, small=False):
                n = 1
                for d in dst.shape:
                    n *= d
                cd = 80 + 0.0090 * n   # DVE estimated ns
                ca = 250 + 0.0066 * n  # ACT estimated ns
                if eng_load[0] + cd <= eng_load[1] + ca:
                    nc.vector.tensor_copy(out=dst, in_=src)
                    eng_load[0] += cd
                else:
                    nc.scalar.copy(dst, src)
                    eng_load[1] += ca

            # Software-pipelined emission: stage A of work unit t+1 is
            # emitted before stage B of unit t, so the PE always has
            # independent matmuls queued behind each copy dependency and
            # fill/drain of consecutive matmuls overlap.

            # ---- pass 1a: W slots (dense 2-GEMM, warms the PE) ----
            wunits = [(j, c) for j in wslots for c in range(CHANNELS)]
            wt1 = {}
            wot = {}

            def w_stageA(t):
                j, c = wunits[t]
                ct, base = cslice(j)
                xt = xts[j]
                t1_ps = ps1.tile([128, 2, N], f32, tag="t1")
                for mb in range(2):
                    for a in range(2):
                        nc.tensor.matmul(
                            t1_ps[:, mb, :],
                            lhsT=xt[:, c, a, mb * 128 : (mb + 1) * 128],
                            rhs=ct[:, base + a * N : base + (a + 1) * N],
                            start=(a == 0),
                            stop=(a == 1),
                        )
                t1_sb = t1pool.tile([128, 2, N], f16, tag="t1w")
                copy(t1_sb[:], t1_ps[:])
                wt1[t] = t1_sb

            def w_stageB(t):
                j, c = wunits[t]
                ct, base = cslice(j)
                if c == 0:
                    wot[j] = opool.tile([128, CHANNELS, 2, N], f16, tag="ot", name=f"wot{j}")
                t1_sb = wt1.pop(t)
                o_ps = ps2.tile([128, 2, N], f32, tag="o")
                for mb in range(2):
                    for a in range(2):
                        nc.tensor.matmul(
                            o_ps[:, mb, :],
                            lhsT=t1_sb[:, a, mb * 128 : (mb + 1) * 128],
                            rhs=ct[:, base + a * N : base + (a + 1) * N],
                            start=(a == 0),
                            stop=(a == 1),
                        )
                copy(wot[j][:, c], o_ps[:])
                if c == CHANNELS - 1:
                    nc.scalar.dma_start(oview(j), wot.pop(j)[:])
                    xts.pop(j)

            for t in range(len(wunits) + 2):
                if t < len(wunits):
                    w_stageA(t)
                if t >= 2:
                    w_stageB(t - 2)

            # ---- pass 1b: G slots, analysis to C_c (K x K) ----
            csbs = {}
            gt1 = {}

            def g_stageA(u):
                j = gslots[u]
                _, K, _ = slots[j]
                ct, base = cslice(j)
                xt = xts[j]
                t1_sb = t1pool.tile([128, 2, CHANNELS, K], f16, tag="t1g")
                for c in range(CHANNELS):
                    t1_ps = ps1.tile([128, 2, K], f32, tag="t1")
                    for mb in range(2):
                        for a in range(2):
                            nc.tensor.matmul(
                                t1_ps[:, mb, :],
                                lhsT=xt[:, c, a, mb * 128 : (mb + 1) * 128],
                                rhs=ct[:, base + a * K : base + (a + 1) * K],
                                start=(a == 0),
                                stop=(a == 1),
                            )
                    copy(t1_sb[:, :, c, :], t1_ps[:], small=True)
                gt1[u] = t1_sb

            def g_stageB(u):
                j = gslots[u]
                _, K, _ = slots[j]
                ct, base = cslice(j)
                t1_sb = gt1.pop(u)
                c_ps = ps2.tile([K, CHANNELS, K], f32, tag="o")
                for a in range(2):
                    nc.tensor.matmul(
                        c_ps[:, :, :],
                        lhsT=ct[:, base + a * K : base + (a + 1) * K],
                        rhs=t1_sb[:, a, :, :],
                        start=(a == 0),
                        stop=(a == 1),
                    )
                c_sb = cspool.tile([128, CHANNELS, 128], f16, tag="cs")
                nc.gpsimd.memset(c_sb[:], 0.0)
                copy(c_sb[:K, :, :K], c_ps[:], small=True)
                csbs[j] = c_sb
                xts.pop(j)


            # ---- pass 2: G synthesis (dense N=256 streams) + store ----
            gq3 = {}
            gots = {}

            def s_stageA(u):
                j = gslots[u]
                _, K, _ = slots[j]
                ct, base = cslice(j)
                fbase = base + 2 * K
                c_sb = csbs.pop(j)
                q3s = []
                for c in range(CHANNELS):
                    q3_ps = ps1.tile([128, N], f32, tag="t1")
                    nc.tensor.matmul(
                        q3_ps[:, :],
                        lhsT=c_sb[:, c, :],
                        rhs=ct[:, fbase : fbase + N],
                        start=True,
                        stop=True,
                    )
                    q3_sb = q3pool.tile([128, N], f16, tag="q3")
                    copy(q3_sb[:], q3_ps[:], small=True)
                    q3s.append(q3_sb)
                gq3[u] = q3s

            def s_stageB(u):
                j = gslots[u]
                _, K, _ = slots[j]
                ct, base = cslice(j)
                fbase = base + 2 * K
                q3s = gq3.pop(u)
                ot = opool.tile([128, CHANNELS, 2, N], f16, tag="ot")
                late = u >= len(gslots) - 4
                for c in range(CHANNELS):
                    o_ps = ps2.tile([128, 2, N], f32, tag="o")
                    for mb in range(2):
                        nc.tensor.matmul(
                            o_ps[:, mb, :],
                            lhsT=q3s[c][:, mb * 128 : (mb + 1) * 128],
                            rhs=ct[:, fbase : fbase + N],
                            start=True,
                            stop=True,
                        )
                    copy(ot[:, c], o_ps[:])
                    if late:
                        nc.sync.dma_start(oview(j)[:, c], ot[:, c])
                if not late:
                    nc.scalar.dma_start(oview(j), ot[:])

            ng = len(gslots)
            for u in range(ng + 1):
                if u < ng:
                    g_stageA(u)
                if u >= 1:
                    g_stageB(u - 1)
            for u in range(ng + 2):
                if u < ng:
                    s_stageA(u)
                if u >= 2:
                    s_stageB(u - 2)

    nc.compile()
    return nc


def _get_nc(key):
    if key not in _NC_CACHE:
        _NC_CACHE[key] = _build_nc(key)
    return _NC_CACHE[key]


def _dct_consts():
    if "D" not in _CONST_CACHE:
        n = np.arange(N, dtype=np.float64)
        D = np.sqrt(2.0 / N) * np.cos(np.pi * (n[None, :] + 0.5) * n[:, None] / N)
        D[0] *= 1.0 / np.sqrt(2.0)
        _CONST_CACHE["D"] = D
        _CONST_CACHE["freqs"] = np.pi * n / N
    return _CONST_CACHE["D"], _CONST_CACHE["freqs"]


def _e_of(sigma):
    _, freqs = _dct_consts()
    return np.exp(-(freqs**2) * float(sigma) ** 2 / 2.0)


def _ge_block(sigma, K):
    """[128, 2K+N] fp16: G^T block (cols [0,2K)) then F block (rows :K)."""
    D, _ = _dct_consts()
    e = _e_of(sigma)
    se = np.sqrt(e[:K])
    Ge = se[:, None] * D[:K]          # G  [K, N]
    F = se[:, None] * D[:, :K].T      # F  [K, N]
    blk = np.zeros((128, 2 * K + N), dtype=np.float16)
    blk[:, : 2 * K] = (
        Ge.T.reshape(2, 128, K).transpose(1, 0, 2).reshape(128, 2 * K)
    )
    blk[:K, 2 * K :] = F.astype(np.float16)
    return blk


def _w_block(sigma):
    """[128, 2N] fp16: W = (D diag(e) D)^T in [p, a*N + h] layout."""
    D, _ = _dct_consts()
    e = _e_of(sigma)
    W = (D @ (e[:, None] * D)).T
    return np.ascontiguousarray(
        W.reshape(2, 128, N).transpose(1, 0, 2).reshape(128, 2 * N)
    ).astype(np.float16)


def kernel(x, blur_sigmas, fwd_steps):
    global LAST_EXEC_TIME_NS
    from concourse import bass_utils

    x = np.ascontiguousarray(np.asarray(x), dtype=np.float32)
    assert x.shape == (BATCH, CHANNELS, N, N), x.shape
    sig = np.asarray(blur_sigmas, dtype=np.float64)
    steps = np.asarray(fwd_steps).astype(np.int64)

    order, slots = _slot_plan(steps)
    key = tuple(slots)
    proc, xpos, n16, n8, coff, ccols, bounds = _layout(key)
    nc = _get_nc(key)

    geb = {}
    wb = {}
    in_maps = []
    for i in range(N_CORES):
        x16 = np.empty((n16, 128, IM), dtype=np.float16) if n16 else None
        x8 = (
            np.empty((n8, 128, IM), dtype=ml_dtypes.float8_e4m3) if n8 else None
        )
        gw = np.empty((128, ccols), dtype=np.float16)
        for j in proc:
            mode, K, is8 = slots[j]
            b = int(order[8 * j + i])
            s = int(steps[b])
            img = x[b]  # [C, N, N]
            if mode == "G":
                # G chain consumes X^T: lhsT rows = X columns
                img = img.transpose(0, 2, 1)
            packed = (
                np.ascontiguousarray(img)
                .reshape(CHANNELS, 2, 128, N)
                .transpose(2, 0, 1, 3)
                .reshape(128, IM)
            )
            if is8:
                x8[xpos[j]] = packed.astype(ml_dtypes.float8_e4m3)
            else:
                x16[xpos[j]] = packed.astype(np.float16)
            base = coff[j][1]
            if mode == "W":
                if s not in wb:
                    wb[s] = _w_block(sig[s])
                gw[:, base : base + 2 * N] = wb[s]
            else:
                if (s, K) not in geb:
                    geb[(s, K)] = _ge_block(sig[s], K)
                gw[:, base : base + 2 * K + N] = geb[(s, K)]
        m = {"gw": gw}
        if n16:
            m["x16"] = x16
        if n8:
            m["x8"] = x8
        in_maps.append(m)

    trace = os.environ.get("BASS_DCT_TRACE", "0") == "1"
    kwargs = {}
    if trace:
        _install_ntff_hook()
        kwargs["trace"] = True
        tmpdir = os.environ.get("BASS_DCT_TRACE_DIR")
        if tmpdir:
            kwargs["tmpdir"] = tmpdir
    res = None
    for attempt in range(3):
        try:
            res = bass_utils.run_bass_kernel_spmd(
                nc, in_maps, core_ids=list(range(N_CORES)), **kwargs
            )
            break
        except Exception:
            if attempt == 2:
                raise
            import time as _time

            _time.sleep(2.0)
            kwargs.pop("trace", None)
            kwargs.pop("tmpdir", None)
    LAST_EXEC_TIME_NS = res.exec_time_ns

    out = np.empty((BATCH, CHANNELS, N, N), dtype=np.float32)
    for i in range(N_CORES):
        oc = res.results[i]["o"]
        for j in range(NSLOT):
            b = int(order[8 * j + i])
            img = (
                oc[j]
                .reshape(128, CHANNELS, 2, N)
                .transpose(1, 2, 0, 3)
                .reshape(CHANNELS, N, N)
                .astype(np.float32)
            )
            out[b] = img
    return out


# revision 16
# speedup vs baseline: 1.0573x; 1.0573x over previous
"""DCT heat-blur kernel for Trainium2 (8 NeuronCores, Bass/Tile).

Math: reference computes, per image X (one (batch, channel) slice):
    coefs = D X D^T;  coefs *= E;  out = D coefs D^T
with E[h,w] = exp(-(f_h^2 + f_w^2) t_b) = e e^T rank-1.  The elementwise
decay therefore factors through the transforms:
    out = (D diag(e) D) X (D diag(e) D)^T = W^T X W,   W = (D diag(e) D)^T.
W_b is a tiny per-batch 256x256 matrix; the device builds it from e_b
(256 floats/batch) and the DCT matrix:  W = (diag(e) D)^T D^T.
The device then does 2 GEMMs per image instead of 4 + an elementwise pass.

Device layout per 256x256 image: row-blocks a=0,1 of 128 rows each.
apply(A, R)[m,h] = sum_k A[k,m] R[k,h] = (A^T R)[m,h] via
matmul(out[mb], lhsT=A[:, a, mb*128:(mb+1)*128], rhs=R[:, a, :]) summed
over a.  out = apply(apply(X, W), W).

Matmuls run in fp32r (fp32 with 11-bit mantissa, full PE rate) unless
BASS_DCT_MM_DTYPE=float32. fp32r operands must come from "rounded"
producers, so host data is pre-rounded (RNE to 11-bit mantissa) and DMAd
into float32r-typed tiles; device-side producers write float32r outputs.

x / out are pre/post-permuted on host into the exact SBUF layout so all
big DMAs are fully contiguous (8KB per partition per transfer).

Sharding: pure data parallel over batch, 16 batches (48 images) per core.
"""

import os
import numpy as np

BATCH = 128
CHANNELS = 3
N = 256
N_CORES = 8
PB = BATCH // N_CORES          # batches per core
IMGS = PB * CHANNELS           # images per core
GRP = 4                        # images per DMA group (1 MiB transfers)
NG = IMGS // GRP               # groups per core

# set BASS_DCT_MM_DTYPE=float32 to fall back to exact-rate fp32 matmuls
_MM_DTYPE = os.environ.get("BASS_DCT_MM_DTYPE", "float16")

LAST_EXEC_TIME_NS = None
_NC_CACHE = {}


def _round_f32r(a):
    """Round fp32 array to fp32r (11-bit mantissa) with round-to-nearest-even."""
    u = np.ascontiguousarray(a, dtype=np.float32).view(np.uint32)
    bias = np.uint32(0x7FF) + ((u >> np.uint32(12)) & np.uint32(1))
    r = (u + bias) & np.uint32(0xFFFFF000)
    return r.view(np.float32)


def _install_ntff_hook():
    """Wire antenv.axon_hooks (missing in this image) so trace=True works."""
    import sys
    import types

    if "antenv.axon_hooks" in sys.modules:
        return
    try:
        import trn_agent_boot.trn_boot as tb

        hook = tb._ntff_profile_via_ctypes("/opt/axon/libaxon_pjrt.so")
    except Exception:
        hook = None
    m = types.ModuleType("antenv.axon_hooks")
    m.get_axon_ntff_profile_hook = lambda: hook
    m.set_axon_ntff_profile_hook = lambda h: None
    sys.modules["antenv.axon_hooks"] = m


def _build_nc():
    import concourse.bacc as bacc
    import concourse.tile as tile
    import concourse.mybir as mybir

    f32 = mybir.dt.float32
    mm_dt = getattr(mybir.dt, _MM_DTYPE)

    nc = bacc.Bacc("TRN2", target_bir_lowering=False, debug=False)
    # x/o are host-permuted: [group][partition][img_in_grp, rowblk, col]
    x_d = nc.dram_tensor("x", [NG, 128, GRP * 2 * N], mm_dt, kind="ExternalInput").ap()
    # w: host-built per-batch W matrices, [partition][batch, rowblk, col]
    w_d = nc.dram_tensor("w", [128, PB, 2, N], mm_dt, kind="ExternalInput").ap()
    # fp16 output: harness tolerance is 2e-2, fp16 quantization is ~5e-4;
    # halves the store traffic (the kernel is HBM-bound)
    o_dt = mybir.dt.float16
    o_d = nc.dram_tensor("o", [NG, 128, GRP * 2 * N], o_dt, kind="ExternalOutput").ap()

    PREFETCH = NG

    with tile.TileContext(nc) as tc:
        with (
            tc.tile_pool(name="const", bufs=1) as cpool,
            tc.tile_pool(name="apool", bufs=2) as apool,
            tc.tile_pool(name="xpool", bufs=PREFETCH + 1) as xpool,
            tc.tile_pool(name="tpool", bufs=6) as tpool,
            tc.tile_pool(name="opool", bufs=6) as opool,
            tc.tile_pool(name="ps1", bufs=4, space="PSUM") as ps1,
            tc.tile_pool(name="ps2", bufs=4, space="PSUM") as ps2,
        ):
            def ld_ring(g):
                return nc.sync

            # fp16 loads are tiny and fully prefetched on sync; late store
            # issues go via the sync engine/ring (idle once loads finish)
            # so they never wait behind ACT's copy work
            def st_ring(g):
                return nc.scalar if g < NG // 2 else nc.sync

            xt_tiles = {}

            def issue_load(g):
                xt = xpool.tile([128, GRP, 2, N], mm_dt, tag="xt")
                ld_ring(g).dma_start(
                    xt[:], x_d[g].rearrange("p (i a w) -> p i a w", i=GRP, a=2)
                )
                xt_tiles[g] = xt

            # first matmul should wait on as little DMA as possible: its W
            # (one batch, 128KB) and its image (128KB) go first on the sync
            # ring; everything else follows
            w_q = {}
            with tc.high_priority():
                wq0a = cpool.tile([128, 1, 2, N], mm_dt, name="wq0a")
                nc.sync.dma_start(wq0a[:], w_d[:, 0:1])
            x0_tiles = []
            x0_src = x_d[0].rearrange("p (i a w) -> p i a w", i=GRP, a=2)
            for ii in range(GRP):
                x0 = xpool.tile([128, 1, 2, N], mm_dt, name=f"xt0_{ii}", tag="xt0")
                if ii == 0:
                    with tc.high_priority():
                        nc.sync.dma_start(x0[:], x0_src[:, ii : ii + 1])
                else:
                    nc.sync.dma_start(x0[:], x0_src[:, ii : ii + 1])
                x0_tiles.append(x0)
            xt_tiles[0] = x0_tiles
            wq0b = cpool.tile([128, 3, 2, N], mm_dt, name="wq0b")
            nc.sync.dma_start(wq0b[:], w_d[:, 1:4])
            for q in range(1, 4):
                wq = cpool.tile([128, 4, 2, N], mm_dt, name=f"wq{q}")
                nc.scalar.dma_start(wq[:], w_d[:, 4 * q : 4 * (q + 1)])
                w_q[q] = wq

            def w_rhs(b, a):
                if b == 0:
                    return wq0a[:, 0, a, :]
                if b < 4:
                    return wq0b[:, b - 1, a, :]
                return w_q[b // 4][:, b % 4, a, :]

            for g in range(1, PREFETCH):
                issue_load(g)

            for g in range(NG):
                if g + PREFETCH < NG:
                    issue_load(g + PREFETCH)
                xt = xt_tiles.pop(g)
                ot = opool.tile([128, GRP, 2, N], o_dt)
                for ii in range(GRP):
                    img = g * GRP + ii
                    b = img // CHANNELS
                    t1_ps = ps1.tile([128, 2, N], f32)
                    for mb in range(2):
                        for a in range(2):
                            nc.tensor.matmul(
                                t1_ps[:, mb, :],
                                lhsT=(
                                    xt[ii][:, 0, a, mb * 128 : (mb + 1) * 128]
                                    if g == 0
                                    else xt[:, ii, a, mb * 128 : (mb + 1) * 128]
                                ),
                                rhs=w_rhs(b, a),
                                start=(a == 0),
                                stop=(a == 1),
                            )
                    t1_sb = tpool.tile([128, 2, N], mm_dt)
                    if ii % 2 == 0:
                        nc.vector.tensor_copy(out=t1_sb[:], in_=t1_ps[:])
                    else:
                        nc.scalar.copy(t1_sb[:], t1_ps[:])
                    t2_ps = ps2.tile([128, 2, N], f32)
                    for mb in range(2):
                        for a in range(2):
                            nc.tensor.matmul(
                                t2_ps[:, mb, :],
                                lhsT=t1_sb[:, a, mb * 128 : (mb + 1) * 128],
                                rhs=w_rhs(b, a),
                                start=(a == 0),
                                stop=(a == 1),
                            )
                    if ii % 2 == 0:
                        nc.scalar.copy(ot[:, ii], t2_ps[:])
                    else:
                        nc.vector.tensor_copy(out=ot[:, ii], in_=t2_ps[:])
                    # late groups store per image from the idle sync ring so
                    # the final drain is one 256KB piece, not a 1MB group
                    if g >= NG // 2:
                        nc.sync.dma_start(
                            o_d[g].rearrange(
                                "p (i a w) -> p i a w", i=GRP, a=2
                            )[:, ii],
                            ot[:, ii],
                        )
                if g < NG // 2:
                    st_ring(g).dma_start(
                        o_d[g].rearrange("p (i a w) -> p i a w", i=GRP, a=2), ot[:]
                    )

    nc.compile()
    return nc


def _get_nc():
    key = ("nc", _MM_DTYPE)
    if key not in _NC_CACHE:
        _NC_CACHE[key] = _build_nc()
    return _NC_CACHE[key]


def _host_w(blur_sigmas, fwd_steps):
    """Per-batch W_b = (D diag(e_b) D)^T in device layout [128, B_core-sliced]."""
    sig = np.asarray(blur_sigmas, dtype=np.float64)
    steps = np.asarray(fwd_steps).astype(np.int64)
    n = np.arange(N, dtype=np.float64)
    D = np.sqrt(2.0 / N) * np.cos(np.pi * (n[None, :] + 0.5) * n[:, None] / N)
    D[0] *= 1.0 / np.sqrt(2.0)
    freqs = np.pi * n / N
    np_dt = np.float16 if _MM_DTYPE == "float16" else np.float32
    uniq, inv = np.unique(steps, return_inverse=True)
    ms = np.empty((len(uniq), N, N), dtype=np_dt)
    for i, s in enumerate(uniq):
        t = sig[s] ** 2 / 2.0
        e = np.exp(-(freqs**2) * t)
        w = (D @ (e[:, None] * D)).T
        if _MM_DTYPE == "float32r":
            w = _round_f32r(w.astype(np.float32))
        ms[i] = w.astype(np_dt)
    w_all = ms[inv]  # [B, N, N]
    # device layout [128, B, 2, N]: [p, b, a, h] = W_b[a*128+p, h]
    return np.ascontiguousarray(
        w_all.reshape(BATCH, 2, 128, N).transpose(2, 0, 1, 3)
    )


def kernel(x, blur_sigmas, fwd_steps):
    global LAST_EXEC_TIME_NS
    from concourse import bass_utils

    x = np.ascontiguousarray(np.asarray(x), dtype=np.float32)
    assert x.shape == (BATCH, CHANNELS, N, N), x.shape
    if _MM_DTYPE == "float32r":
        x = _round_f32r(x)
    elif _MM_DTYPE == "float16":
        x = x.astype(np.float16)
    w_host = _host_w(blur_sigmas, fwd_steps)

    # device x layout: [core][NG, 128, GRP*2*N]
    # x[img, a*128+p, w] -> xc[g, p, (i, a, w)]
    xp = (
        x.reshape(N_CORES, NG, GRP, 2, 128, N)
        .transpose(0, 1, 4, 2, 3, 5)
        .reshape(N_CORES, NG, 128, GRP * 2 * N)
    )
    in_maps = []
    for i in range(N_CORES):
        in_maps.append(
            {
                "x": np.ascontiguousarray(xp[i]),
                "w": np.ascontiguousarray(w_host[:, i * PB : (i + 1) * PB]),
            }
        )

    nc = _get_nc()
    trace = os.environ.get("BASS_DCT_TRACE", "0") == "1"
    kwargs = {}
    if trace:
        _install_ntff_hook()
        kwargs["trace"] = True
        tmpdir = os.environ.get("BASS_DCT_TRACE_DIR")
        if tmpdir:
            kwargs["tmpdir"] = tmpdir
    res = None
    for attempt in range(3):
        try:
            res = bass_utils.run_bass_kernel_spmd(
                nc, in_maps, core_ids=list(range(N_CORES)), **kwargs
            )
            break
        except Exception:
            # transient NRT_EXEC_UNIT_UNRECOVERABLE has been observed on the
            # first execution of a freshly loaded NEFF; a retry succeeds
            if attempt == 2:
                raise
            import time as _time

            _time.sleep(2.0)
            kwargs.pop("trace", None)
            kwargs.pop("tmpdir", None)
    LAST_EXEC_TIME_NS = res.exec_time_ns

    # inverse permute: oc[g, p, (i, a, w)] -> out[img, a*128+p, w]
    oc = np.stack([res.results[i]["o"] for i in range(N_CORES)])
    out = (
        oc.reshape(N_CORES, NG, 128, GRP, 2, N)
        .transpose(0, 1, 3, 4, 2, 5)
        .reshape(BATCH, CHANNELS, N, N)
    )
    return np.ascontiguousarray(out, dtype=np.float32)

